# revision 2
# baseline (speedup 1.0000x reference)
"""Baseline + unroll + bn-fold matmuls + sigmoid-only activations.

Data-parallel: batch 64 -> 8 cores x 8. Per core:
  encoder GRU loop (S steps), attention, decoder GRU+concat loop (SD steps),
  output projection.

Layouts (per core, B=8 local batch):
  h16/h32   : SBUF (128, 64)  free = k*8+b   (k = h k-tile 0..7)
  psum_g    : PSUM (128, 192) free = m*8+b   (m = gate m-tile 0..23: r 0-7, z 8-15, n 16-23)
  W tiles   : SBUF (128, 24576) fp16, col block (m*8+k)*128 holds lhsT tile
              elem (p, c) = W[m*128+c, k*128+p]
  GI dram   : rows (t, m, b) x 128 p, fp32
  encH dram : rows (t, k, b) x 128 p, fp16
"""
import sys
if '/opt/trn_rl_repo' not in sys.path:
    sys.path.insert(0, '/opt/trn_rl_repo')

import numpy as np
import ml_dtypes
import concourse.bass as bass
import concourse.bacc as bacc
import concourse.mybir as mybir
import concourse.tile as tile
from concourse.bass import ts as bts
from concourse.bass_utils import run_bass_kernel_spmd

FP32 = mybir.dt.float32
FP16 = mybir.dt.float16
AF = mybir.ActivationFunctionType
ALU = mybir.AluOpType
AXX = mybir.AxisListType.X

H = 1024
KT = 8          # h k-tiles
MT = 24         # gate m-tiles
B = 8           # local batch
OUT = 512
P = 128

M_ORDER = list(range(0, 8)) + list(range(16, 24)) + list(range(8, 16))  # r, n, z


def build_nc(S=1024, SD=256, n_gi_splits=4):
    nc = bacc.Bacc("TRN2", target_bir_lowering=False, debug=False, num_devices=8)
    PE = mybir.EngineType.PE

    d_wenc = nc.dram_tensor("wenc", [P, MT * KT * P], FP16, kind="ExternalInput")
    d_wdec = nc.dram_tensor("wdec", [P, MT * KT * P], FP16, kind="ExternalInput")
    d_wch = nc.dram_tensor("wch", [P, 8 * 8 * P], FP16, kind="ExternalInput")
    d_wcc = nc.dram_tensor("wcc", [P, 8 * 8 * P], FP16, kind="ExternalInput")
    d_wout = nc.dram_tensor("wout", [P, 4 * 8 * P], FP16, kind="ExternalInput")
    d_watt = nc.dram_tensor("watt", [P, 8 * 8 * P], FP16, kind="ExternalInput")
    d_attb = nc.dram_tensor("attb", [1, 8 * P], FP16, kind="ExternalInput")
    d_gipre_e = nc.dram_tensor("gipre_e", [2, MT * P], FP16, kind="ExternalInput")
    d_gipre_d = nc.dram_tensor("gipre_d", [2, MT * P], FP16, kind="ExternalInput")
    d_xones_e = nc.dram_tensor("xones_e", [2, S * B], FP16, kind="ExternalInput")
    d_xones_d = nc.dram_tensor("xones_d", [2, SD * B], FP16, kind="ExternalInput")
    d_bn_e = nc.dram_tensor("bn_e", [1, 8 * P], FP16, kind="ExternalInput")
    d_bn_d = nc.dram_tensor("bn_d", [1, 8 * P], FP16, kind="ExternalInput")
    d_cbias = nc.dram_tensor("cbias", [P, 8], FP32, kind="ExternalInput")
    d_outb = nc.dram_tensor("outb", [P, 4], FP32, kind="ExternalInput")
    d_out = nc.dram_tensor("out", [P, 4 * B], FP32, kind="ExternalOutput")

    assert S % n_gi_splits == 0
    S_SP = S // n_gi_splits
    assert S_SP % 64 == 0 and SD % 64 == 0
    d_gie = [nc.dram_tensor(f"gie{i}", [MT * P, S_SP * B], FP16)
             for i in range(n_gi_splits)]
    d_gid = nc.dram_tensor("gid", [MT * P, SD * B], FP16)
    d_encH = nc.dram_tensor("encH", [KT * P, S * B], FP16)
    d_escr = nc.dram_tensor("escr", [1, S * B], FP32)
    d_wscr = nc.dram_tensor("wscr", [1, S * B], FP16)

    NCH = S * B // 512  # 512-wide (t,b) chunks, 64 timesteps each

    with tile.TileContext(nc) as tc:
        with tc.tile_pool(name="wbig", bufs=2) as pw, \
             tc.tile_pool(name="state", bufs=1) as pst, \
             tc.tile_pool(name="small", bufs=1) as psm:

            wenc = pw.tile([P, MT * KT * P], FP16, tag="W")
            nc.sync.dma_start(wenc[:], d_wenc[:])

            h16 = pst.tile([P, KT * B], FP16)
            h32 = pst.tile([P, KT * B], FP32)
            nc.vector.memset(h16[:], 0.0)
            nc.vector.memset(h32[:], 0.0)
            hgru16 = pst.tile([P, KT * B], FP16)
            hgru32 = pst.tile([P, KT * B], FP32)
            ctx16 = pst.tile([P, KT * B], FP16)
            cc32 = pst.tile([P, KT * B], FP32)
            onescol = pst.tile([P, 1], FP16)
            nc.vector.memset(onescol[:], 1.0)
            ones512 = pst.tile([1, 512], FP16)
            nc.vector.memset(ones512[:], 1.0)

            bn_e = psm.tile([1, 8 * P], FP16)
            nc.sync.dma_start(bn_e[:], d_bn_e[:])
            bn_d = psm.tile([1, 8 * P], FP16)
            nc.sync.dma_start(bn_d[:], d_bn_d[:])
            ones8 = psm.tile([1, 8], FP16)
            nc.vector.memset(ones8[:], 1.0)
            cbias = psm.tile([P, 8], FP32)
            nc.sync.dma_start(cbias[:], d_cbias[:])
            outb = psm.tile([P, 4], FP32)
            nc.sync.dma_start(outb[:], d_outb[:])

            # ================= GI prologue =================
            def gi_prologue(d_xones, d_gipre, d_gi_list, steps):
                n_chunks = steps * B // 512
                per = n_chunks // len(d_gi_list)
                with tc.tile_pool(name="gipro", bufs=1) as pp, \
                     tc.tile_pool(name="gipsum", bufs=4, space="PSUM") as pps:
                    xo = pp.tile([2, steps * B], FP16, tag="xo")
                    nc.sync.dma_start(xo[:], d_xones[:])
                    gp = pp.tile([2, MT * P], FP16, tag="gp")
                    nc.sync.dma_start(gp[:], d_gipre[:])
                    for c in range(n_chunks):
                        d_gi = d_gi_list[c // per]
                        c_in = c % per
                        for m in range(MT):
                            ps = pps.tile([P, 512], FP32, tag="ps")
                            nc.tensor.matmul(ps[:], gp[:, bts(m, P)],
                                             xo[:, bts(c, 512)],
                                             start=True, stop=True)
                            sb = pp.tile([P, 512], FP16, tag="sb", bufs=4)
                            nc.vector.tensor_copy(sb[:], ps[:])
                            nc.sync.dma_start(
                                d_gi[bts(m, P), bts(c_in, 512)], sb[:])

            gi_prologue(d_xones_e, d_gipre_e, d_gie, S)
            gi_prologue(d_xones_d, d_gipre_d, [d_gid], SD)

            # ================= GRU step =================
            def gru_step(iv, w, gi_dram, bn, hin16, hin32, hout16, hout32,
                         pgi, pps, ptmp):
                gi = pgi.tile([P, MT * B], FP16, tag="gi")
                giv = gi_dram.rearrange("(m p) (t b) -> p m t b", p=P, b=B)
                src_ = giv[:, :, bass.ds(iv, 1), :].rearrange("p m t b -> p m (t b)")
                nc.sync.dma_start(gi[:].rearrange("p (m b) -> p m b", b=B), src_)
                ps = pps.tile([P, MT * B], FP32, tag="ps")
                for m in M_ORDER:
                    is_n = m >= 16
                    for k in range(KT):
                        nc.tensor.matmul(ps[:, bts(m, B)],
                                         w[:, bts(m * KT + k, P)],
                                         hin16[:, bts(k, B)],
                                         start=(k == 0),
                                         stop=(k == KT - 1 and not is_n))
                    if is_n:
                        nc.tensor.matmul(ps[:, bts(m, B)],
                                         bn[:, bts(m - 16, P)],
                                         ones8[:], start=False, stop=True)
                rt = ptmp.tile([P, 8 * B], FP32, tag="rt")
                nc.vector.tensor_tensor(rt[:], ps[:, 0:64], gi[:, 0:64], ALU.add)
                r32 = ptmp.tile([P, 8 * B], FP32, tag="r32")
                nc.scalar.activation(r32[:], rt[:], AF.Sigmoid)
                narg = ptmp.tile([P, 8 * B], FP32, tag="narg")
                nc.vector.tensor_tensor(narg[:], ps[:, 128:192], r32[:], ALU.mult)
                nt = ptmp.tile([P, 8 * B], FP32, tag="nt")
                nc.vector.tensor_tensor(nt[:], narg[:], gi[:, 128:192], ALU.add)
                n32 = ptmp.tile([P, 8 * B], FP32, tag="n32")
                nc.scalar.activation(n32[:], nt[:], AF.Sigmoid, scale=2.0)
                zt = ptmp.tile([P, 8 * B], FP32, tag="zt")
                nc.vector.tensor_tensor(zt[:], ps[:, 64:128], gi[:, 64:128], ALU.add)
                z32 = ptmp.tile([P, 8 * B], FP32, tag="z32")
                nc.scalar.activation(z32[:], zt[:], AF.Sigmoid)
                n2 = ptmp.tile([P, 8 * B], FP32, tag="n2")
                nc.vector.tensor_scalar(n2[:], n32[:], 2.0, -1.0,
                                        ALU.mult, ALU.add)
                d_ = ptmp.tile([P, 8 * B], FP32, tag="d_")
                nc.vector.tensor_tensor(d_[:], hin32[:], n2[:], ALU.subtract)
                e_ = ptmp.tile([P, 8 * B], FP32, tag="e_")
                nc.vector.tensor_tensor(e_[:], z32[:], d_[:], ALU.mult)
                nc.vector.tensor_tensor(hout32[:], n2[:], e_[:], ALU.add)
                nc.vector.tensor_copy(hout16[:], hout32[:])

            # ================= encoder =================
            with tc.tile_pool(name="egi", bufs=3) as pgi, \
                 tc.tile_pool(name="eps", bufs=2, space="PSUM") as pps, \
                 tc.tile_pool(name="etmp", bufs=2) as ptmp:
                UN = 4
                for sp in range(n_gi_splits):
                    assert S_SP % UN == 0
                    with tc.For_i(0, S_SP // UN, 1, hint_engines=(PE,)) as iv:
                        for u in range(UN):
                            t_ = iv * UN + u
                            gru_step(t_, wenc, d_gie[sp], bn_e,
                                     h16, h32, h16, h32, pgi, pps, ptmp)
                            t_glob = t_ + sp * S_SP
                            encHv = d_encH.rearrange("(k p) (t b) -> p k t b", p=P, b=B)
                            dst = encHv[:, :, bass.ds(t_glob, 1), :].rearrange(
                                "p k t b -> p k (t b)")
                            nc.sync.dma_start(
                                dst, h16[:].rearrange("p (k b) -> p k b", b=B))

            # ================= attention =================
            watt = pw.tile([P, 8 * 8 * P], FP16, tag="W2", bufs=1)
            nc.sync.dma_start(watt[:], d_watt[:])
            attb = psm.tile([1, 8 * P], FP16)
            nc.sync.dma_start(attb[:], d_attb[:])

            with tc.tile_pool(name="attn", bufs=2) as pa, \
                 tc.tile_pool(name="attp", bufs=2) as pP, \
                 tc.tile_pool(name="aps", bufs=4, space="PSUM") as paps, \
                 tc.tile_pool(name="eps2", bufs=2, space="PSUM") as peps:
                for c in range(NCH):
                    enc_sb = pa.tile([P, KT * 512], FP16, tag="enc")
                    for k in range(KT):
                        nc.sync.dma_start(enc_sb[:, bts(k, 512)],
                                          d_encH[bts(k, P), bts(c, 512)])
                    pse = peps.tile([1, 512], FP32, tag="pse")
                    for half in range(2):
                        pstiles = []
                        for mi in range(4):
                            m = half * 4 + mi
                            psA = paps.tile([P, 512], FP32, tag="psA")
                            for k in range(KT):
                                nc.tensor.matmul(psA[:],
                                                 watt[:, bts(m * KT + k, P)],
                                                 enc_sb[:, bts(k, 512)],
                                                 start=(k == 0), stop=False)
                            nc.tensor.matmul(psA[:], attb[:, bts(m, P)],
                                             ones512[:], start=False, stop=True)
                            pstiles.append((m, psA))
                        for (m, psA) in pstiles:
                            Pm = pP.tile([P, 512], FP16, tag="P")
                            nc.vector.tensor_tensor(Pm[:], psA[:],
                                                    enc_sb[:, bts(m, 512)],
                                                    ALU.mult)
                            nc.tensor.matmul(pse[:], onescol[:], Pm[:],
                                             start=(m == 0), stop=(m == 7))
                    esb = pP.tile([1, 512], FP32, tag="esb")
                    nc.vector.tensor_copy(esb[:], pse[:])
                    nc.sync.dma_start(d_escr[:, bts(c, 512)], esb[:])

            with tc.tile_pool(name="smax", bufs=1) as psx, \
                 tc.tile_pool(name="wps", bufs=2, space="PSUM") as pwps, \
                 tc.tile_pool(name="ctxp", bufs=3) as pcx, \
                 tc.tile_pool(name="ctxa", bufs=1) as pca:
                eb = psx.tile([B, S], FP32)
                nc.sync.dma_start(
                    eb[:], d_escr.rearrange("o (t b) -> (o b) t", b=B))
                mx = psx.tile([B, 1], FP32)
                nc.vector.tensor_reduce(mx[:], eb[:], AXX, ALU.max)
                nmx = psx.tile([B, 1], FP32)
                nc.vector.tensor_scalar_mul(nmx[:], mx[:], -1.0)
                ex = psx.tile([B, S], FP32)
                nc.scalar.activation(ex[:], eb[:], AF.Exp, bias=nmx[:])
                sm = psx.tile([B, 1], FP32)
                nc.vector.tensor_reduce(sm[:], ex[:], AXX, ALU.add)
                rsm = psx.tile([B, 1], FP32)
                nc.vector.reciprocal(rsm[:], sm[:])
                w16 = psx.tile([B, S], FP16)
                nc.vector.tensor_scalar_mul(w16[:], ex[:], rsm[:])
                nc.sync.dma_start(
                    d_wscr.rearrange("o (t b) -> (o b) t", b=B), w16[:])
                wrow = psx.tile([1, S * B], FP16)
                nc.sync.dma_start(wrow[:], d_wscr[:])

                ctx32 = pca.tile([P, KT * B], FP32)
                for k in range(KT):
                    parts = pca.tile([P, NCH * B], FP32, tag="parts")
                    for c in range(NCH):
                        wb = pwps.tile([P, 512], FP32, tag="wb")
                        nc.tensor.matmul(wb[:], ones512[:, 0:P],
                                         wrow[:, bts(c, 512)],
                                         start=True, stop=True)
                        enc_k = pcx.tile([P, 512], FP16, tag="enck")
                        nc.sync.dma_start(enc_k[:],
                                          d_encH[bts(k, P), bts(c, 512)])
                        P2 = pcx.tile([P, 512], FP32, tag="P2")
                        nc.vector.tensor_tensor(P2[:], enc_k[:], wb[:], ALU.mult)
                        nc.vector.tensor_reduce(
                            parts[:, bts(c, B)],
                            P2[:].rearrange("p (t b) -> p b t", b=B),
                            AXX, ALU.add)
                    nc.vector.tensor_reduce(
                        ctx32[:, bts(k, B)],
                        parts[:].rearrange("p (c b) -> p b c", b=B),
                        AXX, ALU.add)
                nc.vector.tensor_copy(ctx16[:], ctx32[:])

            wdec = pw.tile([P, MT * KT * P], FP16, tag="W")
            nc.sync.dma_start(wdec[:], d_wdec[:])

            with tc.tile_pool(name="ccw", bufs=1) as pcc, \
                 tc.tile_pool(name="ccps", bufs=1, space="PSUM") as pccp:
                wcc = pcc.tile([P, 8 * 8 * P], FP16)
                nc.sync.dma_start(wcc[:], d_wcc[:])
                psc = pccp.tile([P, KT * B], FP32)
                for m in range(8):
                    for k in range(KT):
                        nc.tensor.matmul(psc[:, bts(m, B)],
                                         wcc[:, bts(m * KT + k, P)],
                                         ctx16[:, bts(k, B)],
                                         start=(k == 0), stop=(k == KT - 1))
                for m in range(8):
                    nc.vector.tensor_scalar(cc32[:, bts(m, B)], psc[:, bts(m, B)],
                                            cbias[:, m:m + 1], None, ALU.add)

            # ================= decoder =================
            with tc.tile_pool(name="dgi", bufs=3) as pgi, \
                 tc.tile_pool(name="dps", bufs=2, space="PSUM") as pps, \
                 tc.tile_pool(name="dtmp", bufs=2) as ptmp, \
                 tc.tile_pool(name="dwc", bufs=1) as pwc, \
                 tc.tile_pool(name="dcps", bufs=2, space="PSUM") as pcps:
                wch = pwc.tile([P, 8 * 8 * P], FP16)
                nc.sync.dma_start(wch[:], d_wch[:])
                UND = 4
                assert SD % UND == 0
                with tc.For_i(0, SD // UND, 1, hint_engines=(PE,)) as iv:
                  for u in range(UND):
                    t_ = iv * UND + u
                    gru_step(t_, wdec, d_gid, bn_d,
                             h16, h32, hgru16, hgru32, pgi, pps, ptmp)
                    psc = pcps.tile([P, KT * B], FP32, tag="psc")
                    for m in range(8):
                        for k in range(KT):
                            nc.tensor.matmul(psc[:, bts(m, B)],
                                             wch[:, bts(m * KT + k, P)],
                                             hgru16[:, bts(k, B)],
                                             start=(k == 0), stop=(k == KT - 1))
                    nc.vector.tensor_tensor(h32[:], psc[:], cc32[:], ALU.add)
                    nc.vector.tensor_copy(h16[:], h32[:])

            # ================= output =================
            with tc.tile_pool(name="ow", bufs=1) as pow_, \
                 tc.tile_pool(name="ops", bufs=1, space="PSUM") as pops:
                wout = pow_.tile([P, 4 * 8 * P], FP16)
                nc.sync.dma_start(wout[:], d_wout[:])
                pso = pops.tile([P, 4 * B], FP32)
                for m in range(4):
                    for k in range(KT):
                        nc.tensor.matmul(pso[:, bts(m, B)],
                                         wout[:, bts(m * KT + k, P)],
                                         hgru16[:, bts(k, B)],
                                         start=(k == 0), stop=(k == KT - 1))
                o32 = pow_.tile([P, 4 * B], FP32)
                for m in range(4):
                    nc.vector.tensor_scalar(o32[:, bts(m, B)], pso[:, bts(m, B)],
                                            outb[:, m:m + 1], None, ALU.add)
                nc.sync.dma_start(d_out[:], o32[:])

    nc.finalize()
    return nc


# ================= host-side packing =================

def pack_tiles(W, mt, kt):
    """W (mt*128, kt*128) -> (128, mt*kt*128) fp16 lhsT tile pack."""
    W4 = np.asarray(W, np.float32).reshape(mt, P, kt, P)
    return np.ascontiguousarray(
        W4.transpose(3, 0, 2, 1).reshape(P, mt * kt * P)).astype(np.float16)


def make_in_maps(inputs, S=1024, SD=256):
    f16 = np.float16
    concat_W = np.asarray(inputs["concat_W"], np.float32)

    def gipre(w_ih, b_ih, b_hh):
        w_ih = np.asarray(w_ih, np.float32)
        b_ih = np.asarray(b_ih, np.float32)
        b_hh = np.asarray(b_hh, np.float32)
        c = b_ih.copy()
        c[:2 * H] += b_hh[:2 * H]
        return np.ascontiguousarray(np.stack([w_ih[:, 0], c])).astype(f16)

    shared = dict(
        wenc=pack_tiles(inputs["enc_W_hh"], MT, KT),
        wdec=pack_tiles(inputs["dec_W_hh"], MT, KT),
        wch=pack_tiles(concat_W[:, :H], 8, KT),
        wcc=pack_tiles(concat_W[:, H:], 8, KT),
        wout=pack_tiles(inputs["out_W"], 4, KT),
        watt=pack_tiles(inputs["attn_W"], 8, KT),
        attb=np.asarray(inputs["attn_b"], np.float32).reshape(1, 8 * P).astype(f16),
        gipre_e=gipre(inputs["enc_W_ih"], inputs["enc_b_ih"], inputs["enc_b_hh"]),
        gipre_d=gipre(inputs["dec_W_ih"], inputs["dec_b_ih"], inputs["dec_b_hh"]),
        bn_e=np.asarray(inputs["enc_b_hh"], np.float32)[2 * H:]
            .reshape(1, 8 * P).astype(f16),
        bn_d=np.asarray(inputs["dec_b_hh"], np.float32)[2 * H:]
            .reshape(1, 8 * P).astype(f16),
        cbias=np.ascontiguousarray(
            np.asarray(inputs["concat_b"], np.float32).reshape(8, P).T),
        outb=np.ascontiguousarray(
            np.asarray(inputs["out_b"], np.float32).reshape(4, P).T),
    )
    x_e = np.asarray(inputs["input_ftrs"], np.float32)[:, :, 0]
    x_d = np.asarray(inputs["output_ftrs"], np.float32)[:, :, 0]
    maps = []
    for core in range(8):
        bsl = slice(core * B, (core + 1) * B)
        xones_e = np.ones((2, S * B), np.float32)
        xones_e[0] = x_e[bsl].T.reshape(-1)
        xones_d = np.ones((2, SD * B), np.float32)
        xones_d[0] = x_d[bsl].T.reshape(-1)
        m = dict(shared)
        m["xones_e"] = xones_e.astype(f16)
        m["xones_d"] = xones_d.astype(f16)
        maps.append(m)
    return maps


def unpack_out(results):
    full = np.zeros((64, OUT), np.float32)
    for core, r in enumerate(results):
        o = r["out"]
        full[core * B:(core + 1) * B] = (
            o.reshape(P, 4, B).transpose(2, 1, 0).reshape(B, OUT))
    return full


# ================= public entry point =================
_NC_CACHE = {}


def _get_nc():
    if "nc" not in _NC_CACHE:
        _NC_CACHE["nc"] = build_nc(S=1024, SD=256, n_gi_splits=4)
    return _NC_CACHE["nc"]


def run(inputs, trace=False):
    nc = _get_nc()
    maps = make_in_maps(inputs, S=1024, SD=256)
    kw = {}
    if trace:
        _install_ntff_shim()
        kw["trace"] = True
    res = run_bass_kernel_spmd(nc, maps, core_ids=list(range(8)), **kw)
    return unpack_out(res.results), res


def _install_ntff_shim():
    import types
    try:
        from antenv.axon_hooks import get_axon_ntff_profile_hook  # noqa
        return
    except ImportError:
        pass
    import antenv
    mod = types.ModuleType("antenv.axon_hooks")
    mod._hook = None
    def set_axon_ntff_profile_hook(h):
        mod._hook = h
    def get_axon_ntff_profile_hook():
        return mod._hook
    mod.set_axon_ntff_profile_hook = set_axon_ntff_profile_hook
    mod.get_axon_ntff_profile_hook = get_axon_ntff_profile_hook
    sys.modules["antenv.axon_hooks"] = mod
    try:
        from trn_agent_boot.trn_boot import _ntff_profile_via_ctypes
        hook = _ntff_profile_via_ctypes("/opt/axon/libaxon_pjrt.so")
        if hook is not None:
            set_axon_ntff_profile_hook(hook)
    except Exception:
        pass


def kernel(**inputs):
    out, _ = run(inputs, trace=False)
    return out.astype(np.float32)



# revision 3
# speedup vs baseline: 1.0040x; 1.0040x over previous
"""v3: + split r/z/n PSUM tiles so gate epilogue overlaps later MM groups.

Data-parallel: batch 64 -> 8 cores x 8. Per core:
  encoder GRU loop (S steps), attention, decoder GRU+concat loop (SD steps),
  output projection.

Layouts (per core, B=8 local batch):
  h16/h32   : SBUF (128, 64)  free = k*8+b   (k = h k-tile 0..7)
  psum_g    : PSUM (128, 192) free = m*8+b   (m = gate m-tile 0..23: r 0-7, z 8-15, n 16-23)
  W tiles   : SBUF (128, 24576) fp16, col block (m*8+k)*128 holds lhsT tile
              elem (p, c) = W[m*128+c, k*128+p]
  GI dram   : rows (t, m, b) x 128 p, fp32
  encH dram : rows (t, k, b) x 128 p, fp16
"""
import sys
if '/opt/trn_rl_repo' not in sys.path:
    sys.path.insert(0, '/opt/trn_rl_repo')

import numpy as np
import ml_dtypes
import concourse.bass as bass
import concourse.bacc as bacc
import concourse.mybir as mybir
import concourse.tile as tile
from concourse.bass import ts as bts
from concourse.bass_utils import run_bass_kernel_spmd

FP32 = mybir.dt.float32
FP16 = mybir.dt.float16
AF = mybir.ActivationFunctionType
ALU = mybir.AluOpType
AXX = mybir.AxisListType.X

H = 1024
KT = 8          # h k-tiles
MT = 24         # gate m-tiles
B = 8           # local batch
OUT = 512
P = 128

M_ORDER = list(range(0, 8)) + list(range(16, 24)) + list(range(8, 16))  # r, n, z


def build_nc(S=1024, SD=256, n_gi_splits=4):
    nc = bacc.Bacc("TRN2", target_bir_lowering=False, debug=False, num_devices=8)
    PE = mybir.EngineType.PE

    d_wenc = nc.dram_tensor("wenc", [P, MT * KT * P], FP16, kind="ExternalInput")
    d_wdec = nc.dram_tensor("wdec", [P, MT * KT * P], FP16, kind="ExternalInput")
    d_wch = nc.dram_tensor("wch", [P, 8 * 8 * P], FP16, kind="ExternalInput")
    d_wcc = nc.dram_tensor("wcc", [P, 8 * 8 * P], FP16, kind="ExternalInput")
    d_wout = nc.dram_tensor("wout", [P, 4 * 8 * P], FP16, kind="ExternalInput")
    d_watt = nc.dram_tensor("watt", [P, 8 * 8 * P], FP16, kind="ExternalInput")
    d_attb = nc.dram_tensor("attb", [1, 8 * P], FP16, kind="ExternalInput")
    d_gipre_e = nc.dram_tensor("gipre_e", [2, MT * P], FP16, kind="ExternalInput")
    d_gipre_d = nc.dram_tensor("gipre_d", [2, MT * P], FP16, kind="ExternalInput")
    d_xones_e = nc.dram_tensor("xones_e", [2, S * B], FP16, kind="ExternalInput")
    d_xones_d = nc.dram_tensor("xones_d", [2, SD * B], FP16, kind="ExternalInput")
    d_bn_e = nc.dram_tensor("bn_e", [1, 8 * P], FP16, kind="ExternalInput")
    d_bn_d = nc.dram_tensor("bn_d", [1, 8 * P], FP16, kind="ExternalInput")
    d_cbias = nc.dram_tensor("cbias", [P, 8], FP32, kind="ExternalInput")
    d_outb = nc.dram_tensor("outb", [P, 4], FP32, kind="ExternalInput")
    d_out = nc.dram_tensor("out", [P, 4 * B], FP32, kind="ExternalOutput")

    assert S % n_gi_splits == 0
    S_SP = S // n_gi_splits
    assert S_SP % 64 == 0 and SD % 64 == 0
    d_gie = [nc.dram_tensor(f"gie{i}", [MT * P, S_SP * B], FP16)
             for i in range(n_gi_splits)]
    d_gid = nc.dram_tensor("gid", [MT * P, SD * B], FP16)
    d_encH = nc.dram_tensor("encH", [KT * P, S * B], FP16)
    d_escr = nc.dram_tensor("escr", [1, S * B], FP32)
    d_wscr = nc.dram_tensor("wscr", [1, S * B], FP16)

    NCH = S * B // 512  # 512-wide (t,b) chunks, 64 timesteps each

    with tile.TileContext(nc) as tc:
        with tc.tile_pool(name="wbig", bufs=2) as pw, \
             tc.tile_pool(name="state", bufs=1) as pst, \
             tc.tile_pool(name="small", bufs=1) as psm:

            wenc = pw.tile([P, MT * KT * P], FP16, tag="W")
            nc.sync.dma_start(wenc[:], d_wenc[:])

            h16 = pst.tile([P, KT * B], FP16)
            h32 = pst.tile([P, KT * B], FP32)
            nc.vector.memset(h16[:], 0.0)
            nc.vector.memset(h32[:], 0.0)
            hgru16 = pst.tile([P, KT * B], FP16)
            hgru32 = pst.tile([P, KT * B], FP32)
            ctx16 = pst.tile([P, KT * B], FP16)
            cc32 = pst.tile([P, KT * B], FP32)
            onescol = pst.tile([P, 1], FP16)
            nc.vector.memset(onescol[:], 1.0)
            ones512 = pst.tile([1, 512], FP16)
            nc.vector.memset(ones512[:], 1.0)

            bn_e = psm.tile([1, 8 * P], FP16)
            nc.sync.dma_start(bn_e[:], d_bn_e[:])
            bn_d = psm.tile([1, 8 * P], FP16)
            nc.sync.dma_start(bn_d[:], d_bn_d[:])
            ones8 = psm.tile([1, 8], FP16)
            nc.vector.memset(ones8[:], 1.0)
            cbias = psm.tile([P, 8], FP32)
            nc.sync.dma_start(cbias[:], d_cbias[:])
            outb = psm.tile([P, 4], FP32)
            nc.sync.dma_start(outb[:], d_outb[:])

            # ================= GI prologue =================
            def gi_prologue(d_xones, d_gipre, d_gi_list, steps):
                n_chunks = steps * B // 512
                per = n_chunks // len(d_gi_list)
                with tc.tile_pool(name="gipro", bufs=1) as pp, \
                     tc.tile_pool(name="gipsum", bufs=4, space="PSUM") as pps:
                    xo = pp.tile([2, steps * B], FP16, tag="xo")
                    nc.sync.dma_start(xo[:], d_xones[:])
                    gp = pp.tile([2, MT * P], FP16, tag="gp")
                    nc.sync.dma_start(gp[:], d_gipre[:])
                    for c in range(n_chunks):
                        d_gi = d_gi_list[c // per]
                        c_in = c % per
                        for m in range(MT):
                            ps = pps.tile([P, 512], FP32, tag="ps")
                            nc.tensor.matmul(ps[:], gp[:, bts(m, P)],
                                             xo[:, bts(c, 512)],
                                             start=True, stop=True)
                            sb = pp.tile([P, 512], FP16, tag="sb", bufs=4)
                            nc.vector.tensor_copy(sb[:], ps[:])
                            nc.sync.dma_start(
                                d_gi[bts(m, P), bts(c_in, 512)], sb[:])

            gi_prologue(d_xones_e, d_gipre_e, d_gie, S)
            gi_prologue(d_xones_d, d_gipre_d, [d_gid], SD)

            # ================= GRU step =================
            def gru_step(iv, w, gi_dram, bn, hin16, hin32, hout16, hout32,
                         pgi, pps, ptmp):
                gi = pgi.tile([P, MT * B], FP16, tag="gi")
                giv = gi_dram.rearrange("(m p) (t b) -> p m t b", p=P, b=B)
                src_ = giv[:, :, bass.ds(iv, 1), :].rearrange("p m t b -> p m (t b)")
                nc.sync.dma_start(gi[:].rearrange("p (m b) -> p m b", b=B), src_)
                ps_r = pps.tile([P, 8 * B], FP32, tag="psr")
                ps_z = pps.tile([P, 8 * B], FP32, tag="psz")
                ps_n = pps.tile([P, 8 * B], FP32, tag="psn")
                for m in M_ORDER:
                    is_n = m >= 16
                    tgt = ps_r if m < 8 else (ps_z if m < 16 else ps_n)
                    col = m % 8
                    for k in range(KT):
                        nc.tensor.matmul(tgt[:, bts(col, B)],
                                         w[:, bts(m * KT + k, P)],
                                         hin16[:, bts(k, B)],
                                         start=(k == 0),
                                         stop=(k == KT - 1 and not is_n))
                    if is_n:
                        nc.tensor.matmul(tgt[:, bts(col, B)],
                                         bn[:, bts(m - 16, P)],
                                         ones8[:], start=False, stop=True)
                rt = ptmp.tile([P, 8 * B], FP32, tag="rt")
                nc.vector.tensor_tensor(rt[:], ps_r[:], gi[:, 0:64], ALU.add)
                r32 = ptmp.tile([P, 8 * B], FP32, tag="r32")
                nc.scalar.activation(r32[:], rt[:], AF.Sigmoid)
                narg = ptmp.tile([P, 8 * B], FP32, tag="narg")
                nc.vector.tensor_tensor(narg[:], ps_n[:], r32[:], ALU.mult)
                nt = ptmp.tile([P, 8 * B], FP32, tag="nt")
                nc.vector.tensor_tensor(nt[:], narg[:], gi[:, 128:192], ALU.add)
                n32 = ptmp.tile([P, 8 * B], FP32, tag="n32")
                nc.scalar.activation(n32[:], nt[:], AF.Sigmoid, scale=2.0)
                zt = ptmp.tile([P, 8 * B], FP32, tag="zt")
                nc.vector.tensor_tensor(zt[:], ps_z[:], gi[:, 64:128], ALU.add)
                z32 = ptmp.tile([P, 8 * B], FP32, tag="z32")
                nc.scalar.activation(z32[:], zt[:], AF.Sigmoid)
                n2 = ptmp.tile([P, 8 * B], FP32, tag="n2")
                nc.vector.tensor_scalar(n2[:], n32[:], 2.0, -1.0,
                                        ALU.mult, ALU.add)
                d_ = ptmp.tile([P, 8 * B], FP32, tag="d_")
                nc.vector.tensor_tensor(d_[:], hin32[:], n2[:], ALU.subtract)
                e_ = ptmp.tile([P, 8 * B], FP32, tag="e_")
                nc.vector.tensor_tensor(e_[:], z32[:], d_[:], ALU.mult)
                nc.vector.tensor_tensor(hout32[:], n2[:], e_[:], ALU.add)
                nc.vector.tensor_copy(hout16[:], hout32[:])

            # ================= encoder =================
            with tc.tile_pool(name="egi", bufs=3) as pgi, \
                 tc.tile_pool(name="eps", bufs=2, space="PSUM") as pps, \
                 tc.tile_pool(name="etmp", bufs=2) as ptmp:
                UN = 4
                for sp in range(n_gi_splits):
                    assert S_SP % UN == 0
                    with tc.For_i(0, S_SP // UN, 1, hint_engines=(PE,)) as iv:
                        for u in range(UN):
                            t_ = iv * UN + u
                            gru_step(t_, wenc, d_gie[sp], bn_e,
                                     h16, h32, h16, h32, pgi, pps, ptmp)
                            t_glob = t_ + sp * S_SP
                            encHv = d_encH.rearrange("(k p) (t b) -> p k t b", p=P, b=B)
                            dst = encHv[:, :, bass.ds(t_glob, 1), :].rearrange(
                                "p k t b -> p k (t b)")
                            nc.sync.dma_start(
                                dst, h16[:].rearrange("p (k b) -> p k b", b=B))

            # ================= attention =================
            watt = pw.tile([P, 8 * 8 * P], FP16, tag="W2", bufs=1)
            nc.sync.dma_start(watt[:], d_watt[:])
            attb = psm.tile([1, 8 * P], FP16)
            nc.sync.dma_start(attb[:], d_attb[:])

            with tc.tile_pool(name="attn", bufs=2) as pa, \
                 tc.tile_pool(name="attp", bufs=2) as pP, \
                 tc.tile_pool(name="aps", bufs=4, space="PSUM") as paps, \
                 tc.tile_pool(name="eps2", bufs=2, space="PSUM") as peps:
                for c in range(NCH):
                    enc_sb = pa.tile([P, KT * 512], FP16, tag="enc")
                    for k in range(KT):
                        nc.sync.dma_start(enc_sb[:, bts(k, 512)],
                                          d_encH[bts(k, P), bts(c, 512)])
                    pse = peps.tile([1, 512], FP32, tag="pse")
                    for half in range(2):
                        pstiles = []
                        for mi in range(4):
                            m = half * 4 + mi
                            psA = paps.tile([P, 512], FP32, tag="psA")
                            for k in range(KT):
                                nc.tensor.matmul(psA[:],
                                                 watt[:, bts(m * KT + k, P)],
                                                 enc_sb[:, bts(k, 512)],
                                                 start=(k == 0), stop=False)
                            nc.tensor.matmul(psA[:], attb[:, bts(m, P)],
                                             ones512[:], start=False, stop=True)
                            pstiles.append((m, psA))
                        for (m, psA) in pstiles:
                            Pm = pP.tile([P, 512], FP16, tag="P")
                            nc.vector.tensor_tensor(Pm[:], psA[:],
                                                    enc_sb[:, bts(m, 512)],
                                                    ALU.mult)
                            nc.tensor.matmul(pse[:], onescol[:], Pm[:],
                                             start=(m == 0), stop=(m == 7))
                    esb = pP.tile([1, 512], FP32, tag="esb")
                    nc.vector.tensor_copy(esb[:], pse[:])
                    nc.sync.dma_start(d_escr[:, bts(c, 512)], esb[:])

            with tc.tile_pool(name="smax", bufs=1) as psx, \
                 tc.tile_pool(name="wps", bufs=2, space="PSUM") as pwps, \
                 tc.tile_pool(name="ctxp", bufs=3) as pcx, \
                 tc.tile_pool(name="ctxa", bufs=1) as pca:
                eb = psx.tile([B, S], FP32)
                nc.sync.dma_start(
                    eb[:], d_escr.rearrange("o (t b) -> (o b) t", b=B))
                mx = psx.tile([B, 1], FP32)
                nc.vector.tensor_reduce(mx[:], eb[:], AXX, ALU.max)
                nmx = psx.tile([B, 1], FP32)
                nc.vector.tensor_scalar_mul(nmx[:], mx[:], -1.0)
                ex = psx.tile([B, S], FP32)
                nc.scalar.activation(ex[:], eb[:], AF.Exp, bias=nmx[:])
                sm = psx.tile([B, 1], FP32)
                nc.vector.tensor_reduce(sm[:], ex[:], AXX, ALU.add)
                rsm = psx.tile([B, 1], FP32)
                nc.vector.reciprocal(rsm[:], sm[:])
                w16 = psx.tile([B, S], FP16)
                nc.vector.tensor_scalar_mul(w16[:], ex[:], rsm[:])
                nc.sync.dma_start(
                    d_wscr.rearrange("o (t b) -> (o b) t", b=B), w16[:])
                wrow = psx.tile([1, S * B], FP16)
                nc.sync.dma_start(wrow[:], d_wscr[:])

                ctx32 = pca.tile([P, KT * B], FP32)
                for k in range(KT):
                    parts = pca.tile([P, NCH * B], FP32, tag="parts")
                    for c in range(NCH):
                        wb = pwps.tile([P, 512], FP32, tag="wb")
                        nc.tensor.matmul(wb[:], ones512[:, 0:P],
                                         wrow[:, bts(c, 512)],
                                         start=True, stop=True)
                        enc_k = pcx.tile([P, 512], FP16, tag="enck")
                        nc.sync.dma_start(enc_k[:],
                                          d_encH[bts(k, P), bts(c, 512)])
                        P2 = pcx.tile([P, 512], FP32, tag="P2")
                        nc.vector.tensor_tensor(P2[:], enc_k[:], wb[:], ALU.mult)
                        nc.vector.tensor_reduce(
                            parts[:, bts(c, B)],
                            P2[:].rearrange("p (t b) -> p b t", b=B),
                            AXX, ALU.add)
                    nc.vector.tensor_reduce(
                        ctx32[:, bts(k, B)],
                        parts[:].rearrange("p (c b) -> p b c", b=B),
                        AXX, ALU.add)
                nc.vector.tensor_copy(ctx16[:], ctx32[:])

            wdec = pw.tile([P, MT * KT * P], FP16, tag="W")
            nc.sync.dma_start(wdec[:], d_wdec[:])

            with tc.tile_pool(name="ccw", bufs=1) as pcc, \
                 tc.tile_pool(name="ccps", bufs=1, space="PSUM") as pccp:
                wcc = pcc.tile([P, 8 * 8 * P], FP16)
                nc.sync.dma_start(wcc[:], d_wcc[:])
                psc = pccp.tile([P, KT * B], FP32)
                for m in range(8):
                    for k in range(KT):
                        nc.tensor.matmul(psc[:, bts(m, B)],
                                         wcc[:, bts(m * KT + k, P)],
                                         ctx16[:, bts(k, B)],
                                         start=(k == 0), stop=(k == KT - 1))
                for m in range(8):
                    nc.vector.tensor_scalar(cc32[:, bts(m, B)], psc[:, bts(m, B)],
                                            cbias[:, m:m + 1], None, ALU.add)

            # ================= decoder =================
            with tc.tile_pool(name="dgi", bufs=3) as pgi, \
                 tc.tile_pool(name="dps", bufs=2, space="PSUM") as pps, \
                 tc.tile_pool(name="dtmp", bufs=2) as ptmp, \
                 tc.tile_pool(name="dwc", bufs=1) as pwc, \
                 tc.tile_pool(name="dcps", bufs=2, space="PSUM") as pcps:
                wch = pwc.tile([P, 8 * 8 * P], FP16)
                nc.sync.dma_start(wch[:], d_wch[:])
                UND = 4
                assert SD % UND == 0
                with tc.For_i(0, SD // UND, 1, hint_engines=(PE,)) as iv:
                  for u in range(UND):
                    t_ = iv * UND + u
                    gru_step(t_, wdec, d_gid, bn_d,
                             h16, h32, hgru16, hgru32, pgi, pps, ptmp)
                    psc = pcps.tile([P, KT * B], FP32, tag="psc")
                    for m in range(8):
                        for k in range(KT):
                            nc.tensor.matmul(psc[:, bts(m, B)],
                                             wch[:, bts(m * KT + k, P)],
                                             hgru16[:, bts(k, B)],
                                             start=(k == 0), stop=(k == KT - 1))
                    nc.vector.tensor_tensor(h32[:], psc[:], cc32[:], ALU.add)
                    nc.vector.tensor_copy(h16[:], h32[:])

            # ================= output =================
            with tc.tile_pool(name="ow", bufs=1) as pow_, \
                 tc.tile_pool(name="ops", bufs=1, space="PSUM") as pops:
                wout = pow_.tile([P, 4 * 8 * P], FP16)
                nc.sync.dma_start(wout[:], d_wout[:])
                pso = pops.tile([P, 4 * B], FP32)
                for m in range(4):
                    for k in range(KT):
                        nc.tensor.matmul(pso[:, bts(m, B)],
                                         wout[:, bts(m * KT + k, P)],
                                         hgru16[:, bts(k, B)],
                                         start=(k == 0), stop=(k == KT - 1))
                o32 = pow_.tile([P, 4 * B], FP32)
                for m in range(4):
                    nc.vector.tensor_scalar(o32[:, bts(m, B)], pso[:, bts(m, B)],
                                            outb[:, m:m + 1], None, ALU.add)
                nc.sync.dma_start(d_out[:], o32[:])

    nc.finalize()
    return nc


# ================= host-side packing =================

def pack_tiles(W, mt, kt):
    """W (mt*128, kt*128) -> (128, mt*kt*128) fp16 lhsT tile pack."""
    W4 = np.asarray(W, np.float32).reshape(mt, P, kt, P)
    return np.ascontiguousarray(
        W4.transpose(3, 0, 2, 1).reshape(P, mt * kt * P)).astype(np.float16)


def make_in_maps(inputs, S=1024, SD=256):
    f16 = np.float16
    concat_W = np.asarray(inputs["concat_W"], np.float32)

    def gipre(w_ih, b_ih, b_hh):
        w_ih = np.asarray(w_ih, np.float32)
        b_ih = np.asarray(b_ih, np.float32)
        b_hh = np.asarray(b_hh, np.float32)
        c = b_ih.copy()
        c[:2 * H] += b_hh[:2 * H]
        return np.ascontiguousarray(np.stack([w_ih[:, 0], c])).astype(f16)

    shared = dict(
        wenc=pack_tiles(inputs["enc_W_hh"], MT, KT),
        wdec=pack_tiles(inputs["dec_W_hh"], MT, KT),
        wch=pack_tiles(concat_W[:, :H], 8, KT),
        wcc=pack_tiles(concat_W[:, H:], 8, KT),
        wout=pack_tiles(inputs["out_W"], 4, KT),
        watt=pack_tiles(inputs["attn_W"], 8, KT),
        attb=np.asarray(inputs["attn_b"], np.float32).reshape(1, 8 * P).astype(f16),
        gipre_e=gipre(inputs["enc_W_ih"], inputs["enc_b_ih"], inputs["enc_b_hh"]),
        gipre_d=gipre(inputs["dec_W_ih"], inputs["dec_b_ih"], inputs["dec_b_hh"]),
        bn_e=np.asarray(inputs["enc_b_hh"], np.float32)[2 * H:]
            .reshape(1, 8 * P).astype(f16),
        bn_d=np.asarray(inputs["dec_b_hh"], np.float32)[2 * H:]
            .reshape(1, 8 * P).astype(f16),
        cbias=np.ascontiguousarray(
            np.asarray(inputs["concat_b"], np.float32).reshape(8, P).T),
        outb=np.ascontiguousarray(
            np.asarray(inputs["out_b"], np.float32).reshape(4, P).T),
    )
    x_e = np.asarray(inputs["input_ftrs"], np.float32)[:, :, 0]
    x_d = np.asarray(inputs["output_ftrs"], np.float32)[:, :, 0]
    maps = []
    for core in range(8):
        bsl = slice(core * B, (core + 1) * B)
        xones_e = np.ones((2, S * B), np.float32)
        xones_e[0] = x_e[bsl].T.reshape(-1)
        xones_d = np.ones((2, SD * B), np.float32)
        xones_d[0] = x_d[bsl].T.reshape(-1)
        m = dict(shared)
        m["xones_e"] = xones_e.astype(f16)
        m["xones_d"] = xones_d.astype(f16)
        maps.append(m)
    return maps


def unpack_out(results):
    full = np.zeros((64, OUT), np.float32)
    for core, r in enumerate(results):
        o = r["out"]
        full[core * B:(core + 1) * B] = (
            o.reshape(P, 4, B).transpose(2, 1, 0).reshape(B, OUT))
    return full


# ================= public entry point =================
_NC_CACHE = {}


def _get_nc():
    if "nc" not in _NC_CACHE:
        _NC_CACHE["nc"] = build_nc(S=1024, SD=256, n_gi_splits=4)
    return _NC_CACHE["nc"]


def run(inputs, trace=False):
    nc = _get_nc()
    maps = make_in_maps(inputs, S=1024, SD=256)
    kw = {}
    if trace:
        _install_ntff_shim()
        kw["trace"] = True
    res = run_bass_kernel_spmd(nc, maps, core_ids=list(range(8)), **kw)
    return unpack_out(res.results), res


def _install_ntff_shim():
    import types
    try:
        from antenv.axon_hooks import get_axon_ntff_profile_hook  # noqa
        return
    except ImportError:
        pass
    import antenv
    mod = types.ModuleType("antenv.axon_hooks")
    mod._hook = None
    def set_axon_ntff_profile_hook(h):
        mod._hook = h
    def get_axon_ntff_profile_hook():
        return mod._hook
    mod.set_axon_ntff_profile_hook = set_axon_ntff_profile_hook
    mod.get_axon_ntff_profile_hook = get_axon_ntff_profile_hook
    sys.modules["antenv.axon_hooks"] = mod
    try:
        from trn_agent_boot.trn_boot import _ntff_profile_via_ctypes
        hook = _ntff_profile_via_ctypes("/opt/axon/libaxon_pjrt.so")
        if hook is not None:
            set_axon_ntff_profile_hook(hook)
    except Exception:
        pass


def kernel(**inputs):
    out, _ = run(inputs, trace=False)
    return out.astype(np.float32)

